# revision 35
# baseline (speedup 1.0000x reference)
"""Causal multi-head attention on 8 Trainium2 NeuronCores.

Problem (hardcoded): x [4, 2048, 1024] fp32, W_qkv [1024, 3072], b_qkv [3072],
W_o [1024, 1024], b_o [1024]; 16 heads, head_dim 64.

Sharding: 8 cores = 4 batches x 2 head-groups (8 heads each). Each core
computes QKV projection for its (batch, head-group), causal attention for its
8 heads, and a partial out-projection [2048, 1024]. Host sums the two
head-group partials per batch and adds b_o.

Kernel strategy (per core, v3):
  - x strip [512, 1024] fp32r -> PE-transpose (1.5 cyc/row), 4 per psum slab
    -> one strided DVE copy -> xT bf16.
  - Q^T/K^T in natural pair layout [128, nsub, s]: partition rows 0-63 =
    head 2*nsub, rows 64-127 = head 2*nsub+1 (projection output layout,
    single bias add, no zero-padding).
  - Attention chunk = (head pair, one sk block): two ROW-TILED K=64 QK
    matmuls (array row groups 0-63 / 64-127 concurrently) -> psA [128,2,512]
    (2 psum banks, double-buffered); ONE batched exp ACTIVATE over both
    banks (N=1024), windowed on causal-diagonal blocks; one bf16 mask
    multiply per diagonal chunk; AV matmuls (lhsT=Vn[128,65], ones col =
    softmax denominator) with partial-N on diagonal blocks.
  - Softmax normalize per pair: reciprocal_approx_fast + rank-1 PE broadcast.
  - Out partial = matmul(lhsT=OT bf16, rhs=W_o bf16), fp32 DMA out.
  - Issue-order software pipelining: projection/transpose units of strip i+1
    and the out-projection of strip i-1 are interleaved between attention
    chunks of strip i (separate psum tag rings so they don't serialize).
All weights bf16 (loaded once); attention bf16; psum accumulation fp32.
"""

import ml_dtypes
import numpy as np

import concourse.bass as bass
from concourse import bacc
import concourse.mybir as mybir
from concourse.bass_utils import run_bass_kernel_spmd
from concourse.tile import TileContext

B, S, D = 4, 2048, 1024
H, HD = 16, 64
G = 2                  # head groups (cores per batch)
HPG = H // G           # 8 heads per core
NG = HPG * HD          # 512 qkv feature columns per core
N_CORES = 8
STRIP = 512            # sq strip width
NSTRIP = S // STRIP    # 4
DS = D // 128          # 8 contraction subtiles for the projections
FP32 = mybir.dt.float32
R32 = mybir.dt.float32r
BF16 = mybir.dt.bfloat16
AF = mybir.ActivationFunctionType


def build_bass(dbg=False):
    nc = bacc.Bacc("TRN2")

    x_d = nc.dram_tensor("x", [S, D], BF16, kind="ExternalInput")
    wq_d = nc.dram_tensor("wq", [D, NG], BF16, kind="ExternalInput")
    wk_d = nc.dram_tensor("wk", [D, NG], BF16, kind="ExternalInput")
    wv_d = nc.dram_tensor("wv", [D, NG], BF16, kind="ExternalInput")
    bqk_d = nc.dram_tensor("bqk", [128, 8], FP32, kind="ExternalInput")
    onesr_d = nc.dram_tensor("onesr", [1, 128], R32, kind="ExternalInput")
    bv_d = nc.dram_tensor("bv", [1, NG], R32, kind="ExternalInput")
    wo_d = nc.dram_tensor("wo", [NG, D], BF16, kind="ExternalInput")
    out_d = nc.dram_tensor("out", [S, D], BF16, kind="ExternalOutput")

    with TileContext(nc) as tc:
        with (
            tc.tile_pool(name="const", bufs=1) as const,
            tc.tile_pool(name="persist", bufs=1) as persist,
            tc.tile_pool(name="work", bufs=2) as work,
            tc.tile_pool(name="psum", bufs=2, space="PSUM") as psum,
        ):
            ones1x128 = const.tile([1, 128], R32, name="ones1x128")
            nc.sync.dma_start(ones1x128, onesr_d[:, :])
            bqk_sb = const.tile([128, 8], FP32, name="bqk_sb")
            nc.sync.dma_start(bqk_sb, bqk_d[:, :])

            wq_sb = const.tile([128, DS, NG], BF16, name="wq_sb")
            nc.sync.dma_start(wq_sb, wq_d[:, :].rearrange("(ds p) n -> p ds n", p=128))
            wk_sb = const.tile([128, DS, NG], BF16, name="wk_sb")
            nc.sync.dma_start(wk_sb, wk_d[:, :].rearrange("(ds p) n -> p ds n", p=128))
            wv_sb = const.tile([128, DS, NG], BF16, name="wv_sb")
            nc.sync.dma_start(wv_sb, wv_d[:, :].rearrange("(ds p) n -> p ds n", p=128))
            bv_sb = const.tile([1, NG], R32, name="bv_sb")
            nc.sync.dma_start(bv_sb, bv_d[:, :])
            wo_sb = const.tile([128, 4, D], BF16, name="wo_sb")
            nc.sync.dma_start(wo_sb, wo_d[:, :].rearrange("(ns p) e -> p ns e", p=128))

            # Persistent K^T (pair layout) and V tiles (both bf16)
            KT = persist.tile([128, 4, S], BF16, name="KT")
            Vn = persist.tile([128, S // 128, HPG, HD + 1], BF16, name="Vn")
            nc.vector.memset(Vn[:, :, :, HD], 1.0)

            xTs = {}
            QTs = {}
            OTs = {}

            def transpose_unit(i, dg, xT):
                def f():
                    s0 = i * STRIP
                    for j in range(4):
                        ds = 4 * dg + j
                        nc.sync.dma_start_transpose(
                            xT[:, ds, :],
                            x_d[s0:s0 + STRIP,
                                ds * 128:(ds + 1) * 128])
                return f

            def qkproj_unit(i, which, nb, xT, QT):
                def f():
                    s0 = i * STRIP
                    w_sb = wq_sb if which == 0 else wk_sb
                    ps = psum.tile([128, STRIP], FP32, name="ps",
                                   tag="pj", bufs=2)
                    for ds in range(DS):
                        nc.tensor.matmul(
                            ps, lhsT=w_sb[:, ds, nb * 128:(nb + 1) * 128],
                            rhs=xT[:, ds],
                            start=(ds == 0), stop=(ds == DS - 1))
                    bcol = bqk_sb[:, 4 * which + nb:4 * which + nb + 1]
                    if which == 0:
                        nc.vector.tensor_scalar_add(QT[:, nb, :], ps, bcol)
                    else:
                        nc.vector.tensor_scalar_add(
                            KT[:, nb, s0:s0 + STRIP], ps, bcol)
                return f

            def vproj_unit(i, st, xT):
                def f():
                    stg = i * 4 + st
                    psv = psum.tile([128, STRIP], FP32, name="psv",
                                    tag="pj", bufs=2)
                    nc.tensor.matmul(psv, lhsT=ones1x128, rhs=bv_sb,
                                     start=True, stop=False)
                    for ds in range(DS):
                        nc.tensor.matmul(
                            psv,
                            lhsT=xT[:, ds, st * 128:(st + 1) * 128],
                            rhs=wv_sb[:, ds],
                            start=False, stop=(ds == DS - 1))
                    nc.vector.tensor_copy(
                        Vn[:, stg, :, 0:HD],
                        psv.rearrange("p (h d) -> p h d", d=HD))
                return f

            def outproj_unit(i, st, OT):
                def f():
                    s0 = i * STRIP
                    ob = work.tile([128, D], BF16, name="ob", tag="ob",
                                   bufs=2)
                    for ec in range(2):
                        pso = psum.tile([128, STRIP], FP32, name="pso",
                                        tag="pj", bufs=2)
                        for ns in range(4):
                            nc.tensor.matmul(
                                pso,
                                lhsT=OT[:, ns, st * 128:(st + 1) * 128],
                                rhs=wo_sb[:, ns, ec * 512:(ec + 1) * 512],
                                start=(ns == 0), stop=(ns == 3))
                        nc.vector.tensor_copy(
                            ob[:, ec * 512:(ec + 1) * 512], pso)
                    nc.sync.dma_start(
                        out_d[s0 + st * 128:s0 + (st + 1) * 128, :], ob)
                return f

            def proj_units(i, preload=True):
                xT = work.tile([128, DS, STRIP], BF16, name="xT", tag="xT",
                               bufs=2)
                QT = work.tile([128, 4, STRIP], BF16, name="QT", tag="QT",
                               bufs=2)
                xTs[i], QTs[i] = xT, QT
                units = []
                for dg in range(2):
                    units.append(transpose_unit(i, dg, xT))
                for nb in range(4):
                    units.append(qkproj_unit(i, 0, nb, xT, QT))
                    units.append(qkproj_unit(i, 1, nb, xT, QT))
                for st in range(4):
                    units.append(vproj_unit(i, st, xT))
                return units

            def attention_units(i):
                """List of closures: per pair, nsk chunk units + normalize."""
                nsk = 4 * (i + 1)
                QT = QTs[i]
                OT = work.tile([128, 4, STRIP], BF16, name="OT", tag="OT",
                               bufs=2)
                OTs[i] = OT
                pair_lists = []
                for nsub in range(4):
                    state = {}

                    def w0_of(sk, i=i):
                        return max(0, (sk - 4 * i) * 128)

                    def do_qk(sk, nsub=nsub, state=state, w0_of=w0_of):
                        w0 = w0_of(sk)
                        psA = psum.tile([128, 2, STRIP], FP32, name="psA",
                                        tag="psA", bufs=2)
                        state[("psA", sk)] = psA
                        for half in range(2):
                            nc.tensor.matmul(
                                psA[:, half, w0:],
                                lhsT=KT[64 * half:64 * (half + 1), nsub,
                                        sk * 128:(sk + 1) * 128],
                                rhs=QT[64 * half:64 * (half + 1), nsub, w0:],
                                start=True, stop=True)

                    def do_exp(sk, state=state, w0_of=w0_of):
                        w0 = w0_of(sk)
                        psA = state.pop(("psA", sk))
                        expA = work.tile([128, 2, STRIP], BF16, name="expA",
                                         tag="expA", bufs=4)
                        state[("expA", sk)] = expA
                        nc.scalar.activation(expA[:, :, w0:], psA[:, :, w0:],
                                             AF.Exp, scale=0.125)

                    def do_mask(sk, i=i, state=state, w0_of=w0_of):
                        j = sk - 4 * i
                        if j < 0:
                            return
                        w0 = w0_of(sk)
                        expA = state[("expA", sk)]
                        # zero where f_rel < p  (iota = f_rel - p)
                        nc.gpsimd.affine_select(
                            out=expA[:, :, w0:], in_=expA[:, :, w0:],
                            pattern=[[0, 2], [1, STRIP - w0]],
                            compare_op=mybir.AluOpType.is_ge,
                            fill=0.0, base=0, channel_multiplier=-1)

                    def do_av(sk, nsub=nsub, state=state, w0_of=w0_of,
                              nsk=nsk):
                        w0 = w0_of(sk)
                        expA = state.pop(("expA", sk))
                        psO = state["psO"]
                        for half in range(2):
                            h = 2 * nsub + half
                            nc.tensor.matmul(
                                psO[half][0:HD + 1, w0:],
                                lhsT=Vn[:, sk, h, :],
                                rhs=expA[:, half, w0:],
                                start=(sk == 0), stop=(sk == nsk - 1))

                    def finish_pair(state=state):
                        # move psO to SBUF so the pair's psum banks free up
                        # and the normalize can run off the critical path
                        psO = state.pop("psO")
                        state["OC"] = []
                        state["DEN"] = []
                        for half in range(2):
                            oc = work.tile([HD, STRIP], FP32, name="oc",
                                           tag="oc", bufs=4)
                            nc.vector.tensor_copy(oc, psO[half][0:HD, :])
                            den1 = work.tile([1, STRIP], FP32, name="den1",
                                             tag="den1", bufs=4)
                            nc.vector.tensor_copy(
                                den1, psO[half][HD:HD + 1, :])
                            state["OC"].append(oc)
                            state["DEN"].append(den1)

                    def prologue(state=state, do_qk=do_qk, do_exp=do_exp,
                                 do_mask=do_mask, nsk=nsk):
                        state["psO"] = [
                            psum.tile([128, STRIP], FP32, name="psO",
                                      tag="psO"),
                            psum.tile([128, STRIP], FP32, name="psO",
                                      tag="psO"),
                        ]
                        do_qk(0)
                        if nsk > 1:
                            do_qk(1)
                        do_exp(0)
                        do_mask(0)
                        if nsk > 2:
                            do_qk(2)

                    def stage(c, do_qk=do_qk, do_exp=do_exp,
                              do_mask=do_mask, do_av=do_av, nsk=nsk):
                        # software-pipelined: exp/mask one chunk ahead of AV
                        if c + 1 < nsk:
                            do_exp(c + 1)
                            do_mask(c + 1)
                        if c + 3 < nsk:
                            do_qk(c + 3)
                        do_av(c)

                    def normalize(nsub=nsub, state=state):
                        OC = state.pop("OC")
                        DEN = state.pop("DEN")
                        for half in range(2):
                            recip = work.tile([1, STRIP], FP32, name="recip",
                                              tag="recip", bufs=4)
                            nc.vector.reciprocal_approx_fast(
                                recip, DEN[half])
                            recipr = work.tile([1, STRIP], R32, name="recipr",
                                               tag="recipr", bufs=4)
                            nc.vector.tensor_copy(recipr, recip)
                            psB = psum.tile([64, STRIP], FP32, name="psB",
                                            tag="pj", bufs=2)
                            nc.tensor.matmul(psB, lhsT=ones1x128[:, 0:64],
                                             rhs=recipr,
                                             start=True, stop=True)
                            bcast = work.tile([64, STRIP], FP32,
                                              name="bcast", tag="bcast",
                                              bufs=4)
                            nc.vector.tensor_copy(bcast, psB)
                            nc.vector.tensor_mul(
                                OT[64 * half:64 * (half + 1), nsub, :],
                                OC[half][0:HD, :], bcast)

                    plist = [prologue]
                    for c in range(nsk):
                        plist.append(lambda c=c, stage=stage: stage(c))
                    plist.append(finish_pair)
                    pair_lists.append((plist, normalize))
                # defer each pair's normalize until after the next pair's
                # first stage so its psB matmuls never stall the PE queue
                DEFER = True
                units = []
                pending = None
                for plist, norm in pair_lists:
                    units.append(plist[0])
                    units.append(plist[1])
                    if DEFER and pending is not None:
                        units.append(pending)
                    units.extend(plist[2:])
                    if DEFER:
                        pending = norm
                    else:
                        units.append(norm)
                if DEFER:
                    units.append(pending)
                return units

            # ---- prologue: projections for strip 0 ----
            for u in proj_units(0, preload=False):
                u()

            # ---- main pipeline ----
            for i in range(NSTRIP):
                att = attention_units(i)
                fill = []
                if i > 0:
                    for st in range(4):
                        fill.append(outproj_unit(i - 1, st, OTs[i - 1]))
                if i + 1 < NSTRIP:
                    fill.extend(proj_units(i + 1))
                nf, na = len(fill), len(att)
                fi = 0
                for k, u in enumerate(att):
                    u()
                    want = (k + 1) * nf // na
                    while fi < want:
                        fill[fi]()
                        fi += 1
                while fi < nf:
                    fill[fi]()
                    fi += 1

            # ---- tail: out-projection of the last strip ----
            for st in range(4):
                outproj_unit(NSTRIP - 1, st, OTs[NSTRIP - 1])()
    nc.compile()
    return nc


_CACHE = {}


def _causal_masks():
    # mask[p, j, r, f] = 1.0 if f >= 128*j + p else 0  (keep sk <= sq)
    p = np.arange(128)[:, None, None, None]
    j = np.arange(4)[None, :, None, None]
    f = np.arange(STRIP)[None, None, None, :]
    m = (f >= 128 * j + p) & np.ones((1, 1, 2, 1), dtype=bool)
    return m.astype(np.float32).astype(ml_dtypes.bfloat16)


def kernel(x, W_qkv, b_qkv, W_o, b_o):
    x = np.ascontiguousarray(np.asarray(x, dtype=np.float32))
    W_qkv = np.asarray(W_qkv, dtype=np.float32)
    b_qkv = np.asarray(b_qkv, dtype=np.float32)
    W_o = np.asarray(W_o, dtype=np.float32)
    b_o = np.asarray(b_o, dtype=np.float32)

    if "nc" not in _CACHE:
        _CACHE["nc"] = build_bass()
    nc = _CACHE["nc"]

    bf = ml_dtypes.bfloat16
    in_maps = []
    for c in range(N_CORES):
        b, g = c // G, c % G
        n0 = g * NG
        bq = b_qkv[n0:n0 + NG]
        bk = b_qkv[D + n0:D + n0 + NG]
        bqk = np.concatenate(
            [bq.reshape(4, 128).T, bk.reshape(4, 128).T], axis=1)  # [128, 8]
        in_maps.append({
            "x": np.ascontiguousarray(x[b].astype(bf)),
            "wq": np.ascontiguousarray(W_qkv[:, n0:n0 + NG].astype(bf)),
            "wk": np.ascontiguousarray(W_qkv[:, D + n0:D + n0 + NG].astype(bf)),
            "wv": np.ascontiguousarray(
                W_qkv[:, 2 * D + n0:2 * D + n0 + NG].astype(bf)),
            "bqk": np.ascontiguousarray(bqk),
            "bv": np.ascontiguousarray(
                b_qkv[2 * D + n0:2 * D + n0 + NG].reshape(1, NG)),
            "wo": np.ascontiguousarray(W_o[n0:n0 + NG, :].astype(bf)),
            "onesr": np.ones((1, 128), dtype=np.float32),
        })

    _CACHE["in_maps"] = in_maps
    res = run_bass_kernel_spmd(nc, in_maps, list(range(N_CORES)))
    outs = res.results

    out = np.empty((B, S, D), dtype=np.float32)
    for b in range(B):
        out[b] = (outs[G * b]["out"].astype(np.float32)
                  + outs[G * b + 1]["out"].astype(np.float32))
    out += b_o[None, None, :]
    return out


# revision 36
# speedup vs baseline: 1.0349x; 1.0349x over previous
"""Causal multi-head attention on 8 Trainium2 NeuronCores.

Problem (hardcoded): x [4, 2048, 1024] fp32, W_qkv [1024, 3072], b_qkv [3072],
W_o [1024, 1024], b_o [1024]; 16 heads, head_dim 64.

Sharding: 8 cores = 4 batches x 2 head-groups (8 heads each). Each core
computes QKV projection for its (batch, head-group), causal attention for its
8 heads, and a partial out-projection [2048, 1024]. Host sums the two
head-group partials per batch and adds b_o.

Kernel strategy (per core, v3):
  - x strip [512, 1024] fp32r -> PE-transpose (1.5 cyc/row), 4 per psum slab
    -> one strided DVE copy -> xT bf16.
  - Q^T/K^T in natural pair layout [128, nsub, s]: partition rows 0-63 =
    head 2*nsub, rows 64-127 = head 2*nsub+1 (projection output layout,
    single bias add, no zero-padding).
  - Attention chunk = (head pair, one sk block): two ROW-TILED K=64 QK
    matmuls (array row groups 0-63 / 64-127 concurrently) -> psA [128,2,512]
    (2 psum banks, double-buffered); ONE batched exp ACTIVATE over both
    banks (N=1024), windowed on causal-diagonal blocks; one bf16 mask
    multiply per diagonal chunk; AV matmuls (lhsT=Vn[128,65], ones col =
    softmax denominator) with partial-N on diagonal blocks.
  - Softmax normalize per pair: reciprocal_approx_fast + rank-1 PE broadcast.
  - Out partial = matmul(lhsT=OT bf16, rhs=W_o bf16), fp32 DMA out.
  - Issue-order software pipelining: projection/transpose units of strip i+1
    and the out-projection of strip i-1 are interleaved between attention
    chunks of strip i (separate psum tag rings so they don't serialize).
All weights bf16 (loaded once); attention bf16; psum accumulation fp32.
"""

import ml_dtypes
import numpy as np

import concourse.bass as bass
from concourse import bacc
import concourse.mybir as mybir
from concourse.bass_utils import run_bass_kernel_spmd
from concourse.tile import TileContext

B, S, D = 4, 2048, 1024
H, HD = 16, 64
G = 2                  # head groups (cores per batch)
HPG = H // G           # 8 heads per core
NG = HPG * HD          # 512 qkv feature columns per core
N_CORES = 8
STRIP = 512            # sq strip width
NSTRIP = S // STRIP    # 4
DS = D // 128          # 8 contraction subtiles for the projections
FP32 = mybir.dt.float32
R32 = mybir.dt.float32r
BF16 = mybir.dt.bfloat16
AF = mybir.ActivationFunctionType


def build_bass(dbg=False):
    nc = bacc.Bacc("TRN2")

    x_d = nc.dram_tensor("x", [S, D], BF16, kind="ExternalInput")
    wq_d = nc.dram_tensor("wq", [D, NG], BF16, kind="ExternalInput")
    wk_d = nc.dram_tensor("wk", [D, NG], BF16, kind="ExternalInput")
    wv_d = nc.dram_tensor("wv", [D, NG], BF16, kind="ExternalInput")
    bqk_d = nc.dram_tensor("bqk", [128, 8], FP32, kind="ExternalInput")
    onesr_d = nc.dram_tensor("onesr", [1, 128], R32, kind="ExternalInput")
    bv_d = nc.dram_tensor("bv", [1, NG], R32, kind="ExternalInput")
    wo_d = nc.dram_tensor("wo", [NG, D], BF16, kind="ExternalInput")
    out_d = nc.dram_tensor("out", [S, D], BF16, kind="ExternalOutput")

    with TileContext(nc) as tc:
        with (
            tc.tile_pool(name="const", bufs=1) as const,
            tc.tile_pool(name="persist", bufs=1) as persist,
            tc.tile_pool(name="work", bufs=2) as work,
            tc.tile_pool(name="psum", bufs=2, space="PSUM") as psum,
        ):
            ones1x128 = const.tile([1, 128], R32, name="ones1x128")
            nc.sync.dma_start(ones1x128, onesr_d[:, :])
            bqk_sb = const.tile([128, 8], FP32, name="bqk_sb")
            nc.sync.dma_start(bqk_sb, bqk_d[:, :])

            wq_sb = const.tile([128, DS, NG], BF16, name="wq_sb")
            wk_sb = const.tile([128, DS, NG], BF16, name="wk_sb")
            wv_sb = const.tile([128, DS, NG], BF16, name="wv_sb")
            bv_sb = const.tile([1, NG], R32, name="bv_sb")
            wo_sb = const.tile([128, 4, D], BF16, name="wo_sb")

            def load_weights_a():
                nc.sync.dma_start(
                    wq_sb, wq_d[:, :].rearrange("(ds p) n -> p ds n", p=128))

            def load_weights_b():
                nc.sync.dma_start(
                    wk_sb, wk_d[:, :].rearrange("(ds p) n -> p ds n", p=128))
                nc.sync.dma_start(
                    wv_sb, wv_d[:, :].rearrange("(ds p) n -> p ds n", p=128))
                nc.sync.dma_start(bv_sb, bv_d[:, :])

            def load_weights_c():
                nc.sync.dma_start(
                    wo_sb, wo_d[:, :].rearrange("(ns p) e -> p ns e", p=128))

            # Persistent K^T (pair layout) and V tiles (both bf16)
            KT = persist.tile([128, 4, S], BF16, name="KT")
            Vn = persist.tile([128, S // 128, HPG, HD + 1], BF16, name="Vn")
            nc.vector.memset(Vn[:, :, :, HD], 1.0)

            xTs = {}
            QTs = {}
            OTs = {}

            def transpose_unit(i, dg, xT):
                def f():
                    s0 = i * STRIP
                    for j in range(4):
                        ds = 4 * dg + j
                        nc.sync.dma_start_transpose(
                            xT[:, ds, :],
                            x_d[s0:s0 + STRIP,
                                ds * 128:(ds + 1) * 128])
                return f

            def qkproj_unit(i, which, nb, xT, QT):
                def f():
                    s0 = i * STRIP
                    w_sb = wq_sb if which == 0 else wk_sb
                    ps = psum.tile([128, STRIP], FP32, name="ps",
                                   tag="pj", bufs=2)
                    for ds in range(DS):
                        nc.tensor.matmul(
                            ps, lhsT=w_sb[:, ds, nb * 128:(nb + 1) * 128],
                            rhs=xT[:, ds],
                            start=(ds == 0), stop=(ds == DS - 1))
                    bcol = bqk_sb[:, 4 * which + nb:4 * which + nb + 1]
                    if which == 0:
                        nc.vector.tensor_scalar_add(QT[:, nb, :], ps, bcol)
                    else:
                        nc.vector.tensor_scalar_add(
                            KT[:, nb, s0:s0 + STRIP], ps, bcol)
                return f

            def vproj_unit(i, st, xT):
                def f():
                    stg = i * 4 + st
                    psv = psum.tile([128, STRIP], FP32, name="psv",
                                    tag="pj", bufs=2)
                    nc.tensor.matmul(psv, lhsT=ones1x128, rhs=bv_sb,
                                     start=True, stop=False)
                    for ds in range(DS):
                        nc.tensor.matmul(
                            psv,
                            lhsT=xT[:, ds, st * 128:(st + 1) * 128],
                            rhs=wv_sb[:, ds],
                            start=False, stop=(ds == DS - 1))
                    nc.vector.tensor_copy(
                        Vn[:, stg, :, 0:HD],
                        psv.rearrange("p (h d) -> p h d", d=HD))
                return f

            def outproj_unit(i, st, OT):
                def f():
                    s0 = i * STRIP
                    ob = work.tile([128, D], BF16, name="ob", tag="ob",
                                   bufs=2)
                    for ec in range(2):
                        pso = psum.tile([128, STRIP], FP32, name="pso",
                                        tag="pj", bufs=2)
                        for ns in range(4):
                            nc.tensor.matmul(
                                pso,
                                lhsT=OT[:, ns, st * 128:(st + 1) * 128],
                                rhs=wo_sb[:, ns, ec * 512:(ec + 1) * 512],
                                start=(ns == 0), stop=(ns == 3))
                        nc.vector.tensor_copy(
                            ob[:, ec * 512:(ec + 1) * 512], pso)
                    nc.sync.dma_start(
                        out_d[s0 + st * 128:s0 + (st + 1) * 128, :], ob)
                return f

            def proj_units(i):
                xT = work.tile([128, DS, STRIP], BF16, name="xT", tag="xT",
                               bufs=2)
                QT = work.tile([128, 4, STRIP], BF16, name="QT", tag="QT",
                               bufs=2)
                xTs[i], QTs[i] = xT, QT
                units = []
                for dg in range(2):
                    units.append(transpose_unit(i, dg, xT))
                for nb in range(4):
                    units.append(qkproj_unit(i, 0, nb, xT, QT))
                    units.append(qkproj_unit(i, 1, nb, xT, QT))
                for st in range(4):
                    units.append(vproj_unit(i, st, xT))
                return units

            def attention_units(i):
                """List of closures: per pair, nsk chunk units + normalize."""
                nsk = 4 * (i + 1)
                QT = QTs[i]
                OT = work.tile([128, 4, STRIP], BF16, name="OT", tag="OT",
                               bufs=2)
                OTs[i] = OT
                pair_lists = []
                for nsub in range(4):
                    state = {}

                    def w0_of(sk, i=i):
                        return max(0, (sk - 4 * i) * 128)

                    def do_qk(sk, nsub=nsub, state=state, w0_of=w0_of):
                        w0 = w0_of(sk)
                        psA = psum.tile([128, 2, STRIP], FP32, name="psA",
                                        tag="psA", bufs=2)
                        state[("psA", sk)] = psA
                        for half in range(2):
                            nc.tensor.matmul(
                                psA[:, half, w0:],
                                lhsT=KT[64 * half:64 * (half + 1), nsub,
                                        sk * 128:(sk + 1) * 128],
                                rhs=QT[64 * half:64 * (half + 1), nsub, w0:],
                                start=True, stop=True)

                    def do_exp(sk, state=state, w0_of=w0_of):
                        w0 = w0_of(sk)
                        psA = state.pop(("psA", sk))
                        expA = work.tile([128, 2, STRIP], BF16, name="expA",
                                         tag="expA", bufs=4)
                        state[("expA", sk)] = expA
                        nc.scalar.activation(expA[:, :, w0:], psA[:, :, w0:],
                                             AF.Exp, scale=0.125)

                    def do_mask(sk, i=i, state=state, w0_of=w0_of):
                        j = sk - 4 * i
                        if j < 0:
                            return
                        w0 = w0_of(sk)
                        expA = state[("expA", sk)]
                        # zero where f_rel < p  (iota = f_rel - p)
                        nc.gpsimd.affine_select(
                            out=expA[:, :, w0:], in_=expA[:, :, w0:],
                            pattern=[[0, 2], [1, STRIP - w0]],
                            compare_op=mybir.AluOpType.is_ge,
                            fill=0.0, base=0, channel_multiplier=-1)

                    def do_av(sk, nsub=nsub, state=state, w0_of=w0_of,
                              nsk=nsk):
                        w0 = w0_of(sk)
                        expA = state.pop(("expA", sk))
                        psO = state["psO"]
                        for half in range(2):
                            h = 2 * nsub + half
                            nc.tensor.matmul(
                                psO[half][0:HD + 1, w0:],
                                lhsT=Vn[:, sk, h, :],
                                rhs=expA[:, half, w0:],
                                start=(sk == 0), stop=(sk == nsk - 1))

                    def finish_pair(state=state):
                        # move psO to SBUF so the pair's psum banks free up
                        # and the normalize can run off the critical path
                        psO = state.pop("psO")
                        state["OC"] = []
                        state["DEN"] = []
                        for half in range(2):
                            oc = work.tile([HD, STRIP], FP32, name="oc",
                                           tag="oc", bufs=4)
                            nc.vector.tensor_copy(oc, psO[half][0:HD, :])
                            den1 = work.tile([1, STRIP], FP32, name="den1",
                                             tag="den1", bufs=4)
                            nc.vector.tensor_copy(
                                den1, psO[half][HD:HD + 1, :])
                            state["OC"].append(oc)
                            state["DEN"].append(den1)

                    def prologue(state=state, do_qk=do_qk, do_exp=do_exp,
                                 do_mask=do_mask, nsk=nsk):
                        state["psO"] = [
                            psum.tile([128, STRIP], FP32, name="psO",
                                      tag="psO"),
                            psum.tile([128, STRIP], FP32, name="psO",
                                      tag="psO"),
                        ]
                        do_qk(0)
                        if nsk > 1:
                            do_qk(1)
                        do_exp(0)
                        do_mask(0)
                        if nsk > 2:
                            do_qk(2)

                    def stage(c, do_qk=do_qk, do_exp=do_exp,
                              do_mask=do_mask, do_av=do_av, nsk=nsk):
                        # software-pipelined: exp/mask one chunk ahead of AV
                        if c + 1 < nsk:
                            do_exp(c + 1)
                            do_mask(c + 1)
                        if c + 3 < nsk:
                            do_qk(c + 3)
                        do_av(c)

                    def normalize(nsub=nsub, state=state):
                        OC = state.pop("OC")
                        DEN = state.pop("DEN")
                        for half in range(2):
                            recip = work.tile([1, STRIP], FP32, name="recip",
                                              tag="recip", bufs=4)
                            nc.vector.reciprocal_approx_fast(
                                recip, DEN[half])
                            recipr = work.tile([1, STRIP], R32, name="recipr",
                                               tag="recipr", bufs=4)
                            nc.vector.tensor_copy(recipr, recip)
                            psB = psum.tile([64, STRIP], FP32, name="psB",
                                            tag="pj", bufs=2)
                            nc.tensor.matmul(psB, lhsT=ones1x128[:, 0:64],
                                             rhs=recipr,
                                             start=True, stop=True)
                            bcast = work.tile([64, STRIP], FP32,
                                              name="bcast", tag="bcast",
                                              bufs=4)
                            nc.vector.tensor_copy(bcast, psB)
                            nc.vector.tensor_mul(
                                OT[64 * half:64 * (half + 1), nsub, :],
                                OC[half][0:HD, :], bcast)

                    plist = [prologue]
                    for c in range(nsk):
                        plist.append(lambda c=c, stage=stage: stage(c))
                    plist.append(finish_pair)
                    pair_lists.append((plist, normalize))
                # defer each pair's normalize until after the next pair's
                # first stage so its psB matmuls never stall the PE queue
                DEFER = True
                units = []
                pending = None
                for plist, norm in pair_lists:
                    units.append(plist[0])
                    units.append(plist[1])
                    if DEFER and pending is not None:
                        units.append(pending)
                    units.extend(plist[2:])
                    if DEFER:
                        pending = norm
                    else:
                        units.append(norm)
                if DEFER:
                    units.append(pending)
                return units

            # ---- prologue: projections for strip 0 ----
            proj0 = proj_units(0)
            proj0[0]()          # xT ds 0-3 via DMA transpose
            load_weights_a()    # wq
            proj0[1]()          # xT ds 4-7
            load_weights_b()    # wk, wv, bv
            for u in proj0[2:]:
                u()
            load_weights_c()    # wo

            # ---- main pipeline ----
            for i in range(NSTRIP):
                att = attention_units(i)
                fill = []
                if i + 1 < NSTRIP:
                    nxt = proj_units(i + 1)
                    nxt[0]()    # issue next strip's transpose DMAs now so
                    nxt[1]()    # they beat the out-DMAs into the queue
                    fill.extend(nxt[2:])
                if i > 0:
                    for st in range(4):
                        fill.append(outproj_unit(i - 1, st, OTs[i - 1]))
                nf, na = len(fill), len(att)
                fi = 0
                for k, u in enumerate(att):
                    u()
                    want = (k + 1) * nf // na
                    while fi < want:
                        fill[fi]()
                        fi += 1
                while fi < nf:
                    fill[fi]()
                    fi += 1

            # ---- tail: out-projection of the last strip ----
            for st in range(4):
                outproj_unit(NSTRIP - 1, st, OTs[NSTRIP - 1])()
    nc.compile()
    return nc


_CACHE = {}


def _causal_masks():
    # mask[p, j, r, f] = 1.0 if f >= 128*j + p else 0  (keep sk <= sq)
    p = np.arange(128)[:, None, None, None]
    j = np.arange(4)[None, :, None, None]
    f = np.arange(STRIP)[None, None, None, :]
    m = (f >= 128 * j + p) & np.ones((1, 1, 2, 1), dtype=bool)
    return m.astype(np.float32).astype(ml_dtypes.bfloat16)


def kernel(x, W_qkv, b_qkv, W_o, b_o):
    x = np.ascontiguousarray(np.asarray(x, dtype=np.float32))
    W_qkv = np.asarray(W_qkv, dtype=np.float32)
    b_qkv = np.asarray(b_qkv, dtype=np.float32)
    W_o = np.asarray(W_o, dtype=np.float32)
    b_o = np.asarray(b_o, dtype=np.float32)

    if "nc" not in _CACHE:
        _CACHE["nc"] = build_bass()
    nc = _CACHE["nc"]

    bf = ml_dtypes.bfloat16
    in_maps = []
    for c in range(N_CORES):
        b, g = c // G, c % G
        n0 = g * NG
        bq = b_qkv[n0:n0 + NG]
        bk = b_qkv[D + n0:D + n0 + NG]
        bqk = np.concatenate(
            [bq.reshape(4, 128).T, bk.reshape(4, 128).T], axis=1)  # [128, 8]
        in_maps.append({
            "x": np.ascontiguousarray(x[b].astype(bf)),
            "wq": np.ascontiguousarray(W_qkv[:, n0:n0 + NG].astype(bf)),
            "wk": np.ascontiguousarray(W_qkv[:, D + n0:D + n0 + NG].astype(bf)),
            "wv": np.ascontiguousarray(
                W_qkv[:, 2 * D + n0:2 * D + n0 + NG].astype(bf)),
            "bqk": np.ascontiguousarray(bqk),
            "bv": np.ascontiguousarray(
                b_qkv[2 * D + n0:2 * D + n0 + NG].reshape(1, NG)),
            "wo": np.ascontiguousarray(W_o[n0:n0 + NG, :].astype(bf)),
            "onesr": np.ones((1, 128), dtype=np.float32),
        })

    _CACHE["in_maps"] = in_maps
    res = run_bass_kernel_spmd(nc, in_maps, list(range(N_CORES)))
    outs = res.results

    out = np.empty((B, S, D), dtype=np.float32)
    for b in range(B):
        out[b] = (outs[G * b]["out"].astype(np.float32)
                  + outs[G * b + 1]["out"].astype(np.float32))
    out += b_o[None, None, :]
    return out
